# revision 28
# baseline (speedup 1.0000x reference)
import numpy as np
import concourse.bass as bass
import concourse.bacc as bacc
import concourse.tile as tile
import concourse.mybir as mybir
from concourse import bass_utils
from contextlib import ExitStack

B = 4
QL = 1024
HIST = 1024
KVL = 2048
H = 4096
NH = 32
D = 128
T = 4096
NCORES = 8
HPC = NH // NCORES          # 4 heads per core
ROPE_BASE = 10000.0
INV_NORM = 1.0 / float(np.sqrt(D))
NEG = -1.0e30

FP = mybir.dt.float32
FPR = mybir.dt.float32r
AX = mybir.AluOpType
AF = mybir.ActivationFunctionType


def _build():
    nc = bacc.Bacc("TRN2", num_devices=NCORES)
    xT = nc.dram_tensor("xT", [H, T], FPR, kind="ExternalInput")
    w_qk = nc.dram_tensor("w_qk", [H, 2 * HPC * D], FPR, kind="ExternalInput")
    w_v = nc.dram_tensor("w_v", [H, HPC * D], FPR, kind="ExternalInput")
    wd = nc.dram_tensor("wd", [HPC * D, H], FPR, kind="ExternalInput")
    kTh = nc.dram_tensor("kTh", [B, HPC, D, HIST], FPR, kind="ExternalInput")
    vh = nc.dram_tensor("vh", [B, HPC, HIST, D], FPR, kind="ExternalInput")
    cosT = nc.dram_tensor("cosT", [D, T], FP, kind="ExternalInput")
    sinT = nc.dram_tensor("sinT", [D, T], FP, kind="ExternalInput")
    masks = nc.dram_tensor("masks", [4, D, 512], FP, kind="ExternalInput")
    outT = nc.dram_tensor("outT", [H, T], FP, kind="ExternalOutput")

    qkT = nc.dram_tensor("qkT", [2 * HPC * D, T], FPR)  # internal: rope'd q,k (d-major)
    vN = nc.dram_tensor("vN", [T, HPC * D], FPR)        # internal: new v [t, 4h*d]

    with tile.TileContext(nc) as tc, ExitStack() as top:
        cpool = top.enter_context(tc.tile_pool(name="const", bufs=1))
        ones0 = cpool.tile([128, 1], FP)
        nc.vector.memset(ones0[:, :], 1.0)
        ones_col = cpool.tile([128, 1], FPR)
        nc.scalar.copy(ones_col[:, :], ones0[:, :])
        ones0r = cpool.tile([1, 128], FP)
        nc.vector.memset(ones0r[:, :], 1.0)
        ones_row = cpool.tile([1, 128], FPR)
        nc.scalar.copy(ones_row[:, :], ones0r[:, :])

        for b in range(B):
            c0 = b * QL
            # ---------------- stage A: QKV projection (per seq) ----------------
            with ExitStack() as actx:
                psA = actx.enter_context(tc.tile_pool(name=f"psA{b}", bufs=1, space="PSUM"))
                xp = actx.enter_context(tc.tile_pool(name=f"x{b}", bufs=32))
                wqkp = actx.enter_context(tc.tile_pool(name=f"wqk{b}", bufs=12))
                wvp = actx.enter_context(tc.tile_pool(name=f"wv{b}", bufs=3))
                csp = actx.enter_context(tc.tile_pool(name=f"cs{b}", bufs=1))
                qko = actx.enter_context(tc.tile_pool(name=f"qko{b}", bufs=3))
                rotp = actx.enter_context(tc.tile_pool(name=f"rot{b}", bufs=3))
                vop = actx.enter_context(tc.tile_pool(name=f"vo{b}", bufs=3))

                cos_t = csp.tile([128, QL], FP)
                nc.sync.dma_start(out=cos_t[:, :], in_=cosT[:, c0:c0 + QL])
                sin_t = csp.tile([128, QL], FP)
                nc.sync.dma_start(out=sin_t[:, :], in_=sinT[:, c0:c0 + QL])

                x_tiles = []
                for k in range(32):
                    xt = xp.tile([128, QL], FPR)
                    nc.sync.dma_start(out=xt[:, :], in_=xT[k * 128:(k + 1) * 128, c0:c0 + QL])
                    x_tiles.append(xt)

                # A1: q^T,k^T [1024 rows, 1024 cols] with RoPE
                for m in range(8):
                    pst = [psA.tile([128, 512], FP, name=f"pst{i}", bufs=2) for i in range(2)]
                    for k in range(32):
                        wt = wqkp.tile([128, 128], FPR)
                        nc.sync.dma_start(
                            out=wt[:, :],
                            in_=w_qk[k * 128:(k + 1) * 128, m * 128:(m + 1) * 128])
                        for ns in range(2):
                            nc.tensor.matmul(
                                pst[ns][:, :], wt[:, :],
                                x_tiles[k][:, ns * 512:(ns + 1) * 512],
                                start=(k == 0), stop=(k == 31))
                    qk_out = qko.tile([128, QL], FPR)
                    for ns in range(2):
                        sl = slice(ns * 512, (ns + 1) * 512)
                        rot = rotp.tile([128, 512], FP)
                        nc.scalar.mul(rot[0:64, :], pst[ns][64:128, :], -1.0)
                        nc.scalar.copy(rot[64:128, :], pst[ns][0:64, :])
                        nc.vector.tensor_tensor(
                            out=qk_out[:, sl], in0=pst[ns][:, :], in1=cos_t[:, sl], op=AX.mult)
                        nc.vector.tensor_tensor(
                            out=rot[:, :], in0=rot[:, :], in1=sin_t[:, sl], op=AX.mult)
                        nc.vector.tensor_tensor(
                            out=qk_out[:, sl], in0=qk_out[:, sl], in1=rot[:, :], op=AX.add)
                    nc.sync.dma_start(
                        out=qkT[m * 128:(m + 1) * 128, c0:c0 + QL], in_=qk_out[:, :])

                # A2: new V [1024 rows, 512 cols] (t on partitions), 2 halves x 4 psum banks
                for half in range(2):
                    psv = [psA.tile([128, 512], FP, name=f"psv{i}", bufs=1)
                           for i in range(4)]
                    for k in range(32):
                        wvt = wvp.tile([128, 512], FPR)
                        nc.sync.dma_start(out=wvt[:, :], in_=w_v[k * 128:(k + 1) * 128, :])
                        for tt in range(4):
                            ta = half * 4 + tt
                            nc.tensor.matmul(
                                psv[tt][:, :],
                                x_tiles[k][:, ta * 128:(ta + 1) * 128],
                                wvt[:, :],
                                start=(k == 0), stop=(k == 31))
                    for tt in range(4):
                        ta = half * 4 + tt
                        v_sb = vop.tile([128, 512], FPR)
                        nc.scalar.copy(v_sb[:, :], psv[tt][:, :])
                        nc.sync.dma_start(
                            out=vN[c0 + ta * 128:c0 + (ta + 1) * 128, :], in_=v_sb[:, :])

            # ---------------- stages B + C (per seq) ----------------
            with ExitStack() as bctx:
                psB = bctx.enter_context(tc.tile_pool(name=f"psB{b}", bufs=1, space="PSUM"))
                bq = bctx.enter_context(tc.tile_pool(name=f"bq{b}", bufs=2))
                vv = bctx.enter_context(tc.tile_pool(name=f"vv{b}", bufs=36))
                mp = bctx.enter_context(tc.tile_pool(name=f"mp{b}", bufs=4))
                pp = bctx.enter_context(tc.tile_pool(name=f"pp{b}", bufs=4))
                sbp = bctx.enter_context(tc.tile_pool(name=f"sb{b}", bufs=4))
                sml = bctx.enter_context(tc.tile_pool(name=f"sm{b}", bufs=2))
                atp = bctx.enter_context(tc.tile_pool(name=f"at{b}", bufs=9))
                wdp = bctx.enter_context(tc.tile_pool(name=f"wd{b}", bufs=12))
                outp = bctx.enter_context(tc.tile_pool(name=f"out{b}", bufs=4))

                mask_t = []
                for d in range(4):
                    mt = mp.tile([128, 512], FP)
                    nc.sync.dma_start(out=mt[:, :], in_=masks[d, :, :])
                    mask_t.append(mt)

                attn_tiles = [[None, None] for _ in range(HPC)]
                for h in range(HPC):
                    q_t = bq.tile([128, QL], FPR)
                    nc.sync.dma_start(
                        out=q_t[:, :], in_=qkT[h * 128:(h + 1) * 128, c0:c0 + QL])
                    kn_t = bq.tile([128, QL], FPR)
                    nc.sync.dma_start(
                        out=kn_t[:, :],
                        in_=qkT[512 + h * 128: 512 + (h + 1) * 128, c0:c0 + QL])
                    kh_t = bq.tile([128, QL], FPR)
                    nc.sync.dma_start(out=kh_t[:, :], in_=kTh[b, h, :, :])
                    v_tiles = []
                    for t in range(8):
                        vt = vv.tile([128, 128], FPR)
                        nc.sync.dma_start(out=vt[:, :], in_=vh[b, h, t * 128:(t + 1) * 128, :])
                        v_tiles.append(vt)
                    for t in range(8):
                        vt = vv.tile([128, 128], FPR)
                        nc.sync.dma_start(
                            out=vt[:, :],
                            in_=vN[c0 + t * 128:c0 + (t + 1) * 128,
                                   h * 128:(h + 1) * 128])
                        v_tiles.append(vt)

                    for qc in range(2):
                        n_kv = 12 + 4 * qc
                        qsl = slice(qc * 512, (qc + 1) * 512)
                        attn_ps = psB.tile([128, 512], FP, bufs=2)
                        den_ps = psB.tile([1, 512], FP, bufs=1)
                        for ti in range(n_kv):
                            if ti < 8:
                                lk = kh_t[:, ti * 128:(ti + 1) * 128]
                            else:
                                lk = kn_t[:, (ti - 8) * 128:(ti - 7) * 128]
                            s_ps = psB.tile([128, 512], FP, bufs=2)
                            nc.tensor.matmul(
                                s_ps[:, :], lk, q_t[:, qsl],
                                start=True, stop=True)
                            p_t = pp.tile([128, 512], FPR)
                            if ti >= n_kv - 4:
                                s_sb = sbp.tile([128, 512], FP)
                                nc.vector.tensor_tensor(
                                    out=s_sb[:, :], in0=s_ps[:, :],
                                    in1=mask_t[ti - (n_kv - 4)][:, :], op=AX.add)
                                nc.scalar.activation(
                                    p_t[:, :], s_sb[:, :], AF.Exp, scale=INV_NORM)
                            else:
                                nc.scalar.activation(
                                    p_t[:, :], s_ps[:, :], AF.Exp, scale=INV_NORM)
                            nc.tensor.matmul(
                                den_ps[:, :], ones_col[:, :], p_t[:, :],
                                start=(ti == 0), stop=(ti == n_kv - 1))
                            nc.tensor.matmul(
                                attn_ps[:, :], v_tiles[ti][:, :], p_t[:, :],
                                start=(ti == 0), stop=(ti == n_kv - 1))
                        den_sb = sml.tile([1, 512], FP)
                        nc.scalar.copy(den_sb[:, :], den_ps[:, :])
                        rec = sml.tile([1, 512], FPR)
                        with nc.allow_low_precision(reason="fp32r bits are fp32"):
                            nc.vector.reciprocal(rec[:, :], den_sb[:, :])
                        bc_ps = psB.tile([128, 512], FP, bufs=1)
                        nc.tensor.matmul(
                            bc_ps[:, :], ones_row[:, :], rec[:, :],
                            start=True, stop=True)
                        bc_sb = sbp.tile([128, 512], FP)
                        nc.scalar.copy(bc_sb[:, :], bc_ps[:, :])
                        at = atp.tile([128, 512], FPR)
                        nc.vector.tensor_tensor(
                            out=at[:, :], in0=attn_ps[:, :], in1=bc_sb[:, :], op=AX.mult)
                        attn_tiles[h][qc] = at

                # stage C: dense partial out^T for this seq's columns
                for m in range(32):
                    wd_t = []
                    for k in range(HPC):
                        wt = wdp.tile([128, 128], FPR)
                        nc.sync.dma_start(
                            out=wt[:, :],
                            in_=wd[k * 128:(k + 1) * 128, m * 128:(m + 1) * 128])
                        wd_t.append(wt)
                    for qc in range(2):
                        pso = psB.tile([128, 512], FP, bufs=2)
                        for k in range(HPC):
                            nc.tensor.matmul(
                                pso[:, :], wd_t[k][:, :],
                                attn_tiles[k][qc][:, :],
                                start=(k == 0), stop=(k == HPC - 1))
                        o_sb = outp.tile([128, 512], FP)
                        nc.scalar.copy(o_sb[:, :], pso[:, :])
                        nc.sync.dma_start(
                            out=outT[m * 128:(m + 1) * 128,
                                     c0 + qc * 512:c0 + (qc + 1) * 512],
                            in_=o_sb[:, :])

    nc.compile()
    return nc


_NC = None


def _host_prep(hidden_states, w_qkv, w_dense, past_key, past_value,
               block_offsets, position_ids_1d):
    xT = np.ascontiguousarray(np.asarray(hidden_states, np.float32)[0].T)
    w_qkv = np.asarray(w_qkv, np.float32)
    w_dense = np.asarray(w_dense, np.float32)
    bo = np.asarray(block_offsets)
    pos = np.asarray(position_ids_1d)

    inv_freq = (1.0 / (ROPE_BASE ** (np.arange(0, D, 2, dtype=np.float32) / D))).astype(np.float32)
    f2 = np.concatenate([inv_freq, inv_freq]).astype(np.float32)
    ang = pos.astype(np.float32)[None, :] * f2[:, None]          # [128, T]
    cosT = np.cos(ang).astype(np.float32)
    sinT = np.sin(ang).astype(np.float32)

    i = np.arange(128)[:, None]
    j = np.arange(512)[None, :]
    masks = np.stack(
        [np.where(i + 128 * d <= j, np.float32(0.0), np.float32(NEG)) for d in range(4)]
    ).astype(np.float32)                                         # [4, 128, 512]

    nhb = HIST // 64                                             # blocks per seq history
    hist_k = np.asarray(past_key)[bo[:, :nhb]].reshape(B, HIST, NH, D)
    hist_v = np.asarray(past_value)[bo[:, :nhb]].reshape(B, HIST, NH, D)

    wq = w_qkv.reshape(H, NH, 3, D)
    in_maps = []
    for c in range(NCORES):
        hs = slice(c * HPC, (c + 1) * HPC)
        w_qk_c = np.concatenate(
            [wq[:, hs, 0, :].reshape(H, HPC * D), wq[:, hs, 1, :].reshape(H, HPC * D)],
            axis=1)
        w_v_c = wq[:, hs, 2, :].reshape(H, HPC * D)
        wd_c = w_dense[c * HPC * D:(c + 1) * HPC * D, :]
        kTh_c = np.ascontiguousarray(hist_k[:, :, hs, :].transpose(0, 2, 3, 1))
        vh_c = np.ascontiguousarray(hist_v[:, :, hs, :].transpose(0, 2, 1, 3))
        in_maps.append({
            "xT": xT,
            "w_qk": np.ascontiguousarray(w_qk_c),
            "w_v": np.ascontiguousarray(w_v_c),
            "wd": np.ascontiguousarray(wd_c),
            "kTh": kTh_c,
            "vh": vh_c,
            "cosT": cosT,
            "sinT": sinT,
            "masks": masks,
        })
    return in_maps


def kernel(hidden_states, w_qkv, w_dense, past_key, past_value,
           block_offsets, position_ids_1d):
    global _NC
    if _NC is None:
        _NC = _build()
    in_maps = _host_prep(hidden_states, w_qkv, w_dense, past_key, past_value,
                         block_offsets, position_ids_1d)
    res = bass_utils.run_bass_kernel_spmd(_NC, in_maps, core_ids=list(range(NCORES)))
    acc = np.zeros((H, T), np.float32)
    for c in range(NCORES):
        acc += res.results[c]["outT"]
    return np.ascontiguousarray(acc.T).reshape(1, T, H).astype(np.float32)


# revision 29
# speedup vs baseline: 56.3397x; 56.3397x over previous
import numpy as np
import concourse.bass as bass
import concourse.bacc as bacc
import concourse.tile as tile
import concourse.mybir as mybir
from concourse import bass_utils
from contextlib import ExitStack

B = 4
QL = 1024
HIST = 1024
KVL = 2048
H = 4096
NH = 32
D = 128
T = 4096
NCORES = 8
HPC = NH // NCORES          # 4 heads per core
ROPE_BASE = 10000.0
INV_NORM = 1.0 / float(np.sqrt(D))
NEG = -1.0e30

FP = mybir.dt.float32
FPR = mybir.dt.float32r
AX = mybir.AluOpType
AF = mybir.ActivationFunctionType


def _build():
    nc = bacc.Bacc("TRN2", num_devices=NCORES)
    xT = nc.dram_tensor("xT", [H, T], FPR, kind="ExternalInput")
    w_qk = nc.dram_tensor("w_qk", [H, 2 * HPC * D], FPR, kind="ExternalInput")
    w_v = nc.dram_tensor("w_v", [H, HPC * D], FPR, kind="ExternalInput")
    wd = nc.dram_tensor("wd", [HPC * D, H], FPR, kind="ExternalInput")
    kTh = nc.dram_tensor("kTh", [B, HPC, D, HIST], FPR, kind="ExternalInput")
    vh = nc.dram_tensor("vh", [B, HPC, HIST, D], FPR, kind="ExternalInput")
    cosT = nc.dram_tensor("cosT", [D, T], FP, kind="ExternalInput")
    sinT = nc.dram_tensor("sinT", [D, T], FP, kind="ExternalInput")
    masks = nc.dram_tensor("masks", [4, D, 512], FP, kind="ExternalInput")
    outT = nc.dram_tensor("outT", [H, T], FP, kind="ExternalOutput")

    qkT = nc.dram_tensor("qkT", [2 * HPC * D, T], FPR)  # internal: rope'd q,k (d-major)
    vN = nc.dram_tensor("vN", [T, HPC * D], FPR)        # internal: new v [t, 4h*d]

    with tile.TileContext(nc) as tc, ExitStack() as top:
        cpool = top.enter_context(tc.tile_pool(name="const", bufs=1))
        ones0 = cpool.tile([128, 1], FP)
        nc.vector.memset(ones0[:, :], 1.0)
        ones_col = cpool.tile([128, 1], FPR)
        nc.scalar.copy(ones_col[:, :], ones0[:, :])
        ones0r = cpool.tile([1, 128], FP)
        nc.vector.memset(ones0r[:, :], 1.0)
        ones_row = cpool.tile([1, 128], FPR)
        nc.scalar.copy(ones_row[:, :], ones0r[:, :])

        for b in range(B):
            c0 = b * QL
            # ---------------- stage A: QKV projection (per seq) ----------------
            with ExitStack() as actx:
                psA = actx.enter_context(tc.tile_pool(name=f"psA{b}", bufs=1, space="PSUM"))
                xp = actx.enter_context(tc.tile_pool(name=f"x{b}", bufs=32))
                wqkp = actx.enter_context(tc.tile_pool(name=f"wqk{b}", bufs=12))
                wvp = actx.enter_context(tc.tile_pool(name=f"wv{b}", bufs=3))
                csp = actx.enter_context(tc.tile_pool(name=f"cs{b}", bufs=1))
                qko = actx.enter_context(tc.tile_pool(name=f"qko{b}", bufs=3))
                rotp = actx.enter_context(tc.tile_pool(name=f"rot{b}", bufs=3))
                vop = actx.enter_context(tc.tile_pool(name=f"vo{b}", bufs=3))

                cos_t = csp.tile([128, QL], FP)
                nc.sync.dma_start(out=cos_t[:, :], in_=cosT[:, c0:c0 + QL])
                sin_t = csp.tile([128, QL], FP)
                nc.sync.dma_start(out=sin_t[:, :], in_=sinT[:, c0:c0 + QL])

                x_tiles = []
                for k in range(32):
                    xt = xp.tile([128, QL], FPR)
                    nc.sync.dma_start(out=xt[:, :], in_=xT[k * 128:(k + 1) * 128, c0:c0 + QL])
                    x_tiles.append(xt)

                # A1: q^T,k^T [1024 rows, 1024 cols] with RoPE
                for m in range(8):
                    pst = [psA.tile([128, 512], FP, name=f"pst{i}", bufs=2) for i in range(2)]
                    for k in range(32):
                        wt = wqkp.tile([128, 128], FPR)
                        nc.sync.dma_start(
                            out=wt[:, :],
                            in_=w_qk[k * 128:(k + 1) * 128, m * 128:(m + 1) * 128])
                        for ns in range(2):
                            nc.tensor.matmul(
                                pst[ns][:, :], wt[:, :],
                                x_tiles[k][:, ns * 512:(ns + 1) * 512],
                                start=(k == 0), stop=(k == 31))
                    qk_out = qko.tile([128, QL], FPR)
                    for ns in range(2):
                        sl = slice(ns * 512, (ns + 1) * 512)
                        rot = rotp.tile([128, 512], FP)
                        nc.scalar.mul(rot[0:64, :], pst[ns][64:128, :], -1.0)
                        nc.scalar.copy(rot[64:128, :], pst[ns][0:64, :])
                        nc.vector.tensor_tensor(
                            out=qk_out[:, sl], in0=pst[ns][:, :], in1=cos_t[:, sl], op=AX.mult)
                        nc.vector.tensor_tensor(
                            out=rot[:, :], in0=rot[:, :], in1=sin_t[:, sl], op=AX.mult)
                        nc.vector.tensor_tensor(
                            out=qk_out[:, sl], in0=qk_out[:, sl], in1=rot[:, :], op=AX.add)
                    nc.sync.dma_start(
                        out=qkT[m * 128:(m + 1) * 128, c0:c0 + QL], in_=qk_out[:, :])

                # A2: new V [1024 rows, 512 cols] (t on partitions), 2 halves x 4 psum banks
                for half in range(2):
                    psv = [psA.tile([128, 512], FP, name=f"psv{i}", bufs=1)
                           for i in range(4)]
                    for k in range(32):
                        wvt = wvp.tile([128, 512], FPR)
                        nc.sync.dma_start(out=wvt[:, :], in_=w_v[k * 128:(k + 1) * 128, :])
                        for tt in range(4):
                            ta = half * 4 + tt
                            nc.tensor.matmul(
                                psv[tt][:, :],
                                x_tiles[k][:, ta * 128:(ta + 1) * 128],
                                wvt[:, :],
                                start=(k == 0), stop=(k == 31))
                    for tt in range(4):
                        ta = half * 4 + tt
                        v_sb = vop.tile([128, 512], FPR)
                        nc.scalar.copy(v_sb[:, :], psv[tt][:, :])
                        nc.sync.dma_start(
                            out=vN[c0 + ta * 128:c0 + (ta + 1) * 128, :], in_=v_sb[:, :])

            # ---------------- stages B + C (per seq) ----------------
            with ExitStack() as bctx:
                psB = bctx.enter_context(tc.tile_pool(name=f"psB{b}", bufs=1, space="PSUM"))
                bq = bctx.enter_context(tc.tile_pool(name=f"bq{b}", bufs=2))
                vv = bctx.enter_context(tc.tile_pool(name=f"vv{b}", bufs=36))
                mp = bctx.enter_context(tc.tile_pool(name=f"mp{b}", bufs=4))
                pp = bctx.enter_context(tc.tile_pool(name=f"pp{b}", bufs=4))
                sbp = bctx.enter_context(tc.tile_pool(name=f"sb{b}", bufs=4))
                sml = bctx.enter_context(tc.tile_pool(name=f"sm{b}", bufs=2))
                atp = bctx.enter_context(tc.tile_pool(name=f"at{b}", bufs=9))
                wdp = bctx.enter_context(tc.tile_pool(name=f"wd{b}", bufs=12))
                outp = bctx.enter_context(tc.tile_pool(name=f"out{b}", bufs=4))

                mask_t = []
                for d in range(4):
                    mt = mp.tile([128, 512], FP)
                    nc.sync.dma_start(out=mt[:, :], in_=masks[d, :, :])
                    mask_t.append(mt)

                attn_tiles = [[None, None] for _ in range(HPC)]
                for h in range(HPC):
                    q_t = bq.tile([128, QL], FPR)
                    nc.sync.dma_start(
                        out=q_t[:, :], in_=qkT[h * 128:(h + 1) * 128, c0:c0 + QL])
                    kn_t = bq.tile([128, QL], FPR)
                    nc.sync.dma_start(
                        out=kn_t[:, :],
                        in_=qkT[512 + h * 128: 512 + (h + 1) * 128, c0:c0 + QL])
                    kh_t = bq.tile([128, QL], FPR)
                    nc.sync.dma_start(out=kh_t[:, :], in_=kTh[b, h, :, :])
                    v_tiles = []
                    for t in range(8):
                        vt = vv.tile([128, 128], FPR)
                        nc.sync.dma_start(out=vt[:, :], in_=vh[b, h, t * 128:(t + 1) * 128, :])
                        v_tiles.append(vt)
                    for t in range(8):
                        vt = vv.tile([128, 128], FPR)
                        nc.sync.dma_start(
                            out=vt[:, :],
                            in_=vN[c0 + t * 128:c0 + (t + 1) * 128,
                                   h * 128:(h + 1) * 128])
                        v_tiles.append(vt)

                    for qc in range(2):
                        n_kv = 12 + 4 * qc
                        qsl = slice(qc * 512, (qc + 1) * 512)
                        attn_ps = psB.tile([128, 512], FP, bufs=2)
                        den_ps = psB.tile([1, 512], FP, bufs=1)
                        for ti in range(n_kv):
                            if ti < 8:
                                lk = kh_t[:, ti * 128:(ti + 1) * 128]
                            else:
                                lk = kn_t[:, (ti - 8) * 128:(ti - 7) * 128]
                            s_ps = psB.tile([128, 512], FP, bufs=2)
                            nc.tensor.matmul(
                                s_ps[:, :], lk, q_t[:, qsl],
                                start=True, stop=True)
                            p_t = pp.tile([128, 512], FPR)
                            if ti >= n_kv - 4:
                                s_sb = sbp.tile([128, 512], FP)
                                nc.vector.tensor_tensor(
                                    out=s_sb[:, :], in0=s_ps[:, :],
                                    in1=mask_t[ti - (n_kv - 4)][:, :], op=AX.add)
                                nc.scalar.activation(
                                    p_t[:, :], s_sb[:, :], AF.Exp, scale=INV_NORM)
                            else:
                                nc.scalar.activation(
                                    p_t[:, :], s_ps[:, :], AF.Exp, scale=INV_NORM)
                            nc.tensor.matmul(
                                den_ps[:, :], ones_col[:, :], p_t[:, :],
                                start=(ti == 0), stop=(ti == n_kv - 1))
                            nc.tensor.matmul(
                                attn_ps[:, :], v_tiles[ti][:, :], p_t[:, :],
                                start=(ti == 0), stop=(ti == n_kv - 1))
                        den_sb = sml.tile([1, 512], FP)
                        nc.scalar.copy(den_sb[:, :], den_ps[:, :])
                        rec = sml.tile([1, 512], FPR)
                        with nc.allow_low_precision(reason="fp32r bits are fp32"):
                            nc.vector.reciprocal(rec[:, :], den_sb[:, :])
                        bc_ps = psB.tile([128, 512], FP, bufs=1)
                        nc.tensor.matmul(
                            bc_ps[:, :], ones_row[:, :], rec[:, :],
                            start=True, stop=True)
                        bc_sb = sbp.tile([128, 512], FP)
                        nc.scalar.copy(bc_sb[:, :], bc_ps[:, :])
                        at = atp.tile([128, 512], FPR)
                        nc.vector.tensor_tensor(
                            out=at[:, :], in0=attn_ps[:, :], in1=bc_sb[:, :], op=AX.mult)
                        attn_tiles[h][qc] = at

                # stage C: dense partial out^T for this seq's columns
                for m in range(32):
                    wd_t = []
                    for k in range(HPC):
                        wt = wdp.tile([128, 128], FPR)
                        nc.sync.dma_start(
                            out=wt[:, :],
                            in_=wd[k * 128:(k + 1) * 128, m * 128:(m + 1) * 128])
                        wd_t.append(wt)
                    for qc in range(2):
                        pso = psB.tile([128, 512], FP, bufs=2)
                        for k in range(HPC):
                            nc.tensor.matmul(
                                pso[:, :], wd_t[k][:, :],
                                attn_tiles[k][qc][:, :],
                                start=(k == 0), stop=(k == HPC - 1))
                        o_sb = outp.tile([128, 512], FP)
                        nc.scalar.copy(o_sb[:, :], pso[:, :])
                        nc.sync.dma_start(
                            out=outT[m * 128:(m + 1) * 128,
                                     c0 + qc * 512:c0 + (qc + 1) * 512],
                            in_=o_sb[:, :])

    nc.compile()
    return nc


_NC = None


def _host_prep(hidden_states, w_qkv, w_dense, past_key, past_value,
               block_offsets, position_ids_1d):
    xT = np.ascontiguousarray(np.asarray(hidden_states, np.float32)[0].T)
    w_qkv = np.asarray(w_qkv, np.float32)
    w_dense = np.asarray(w_dense, np.float32)
    bo = np.asarray(block_offsets)
    pos = np.asarray(position_ids_1d)

    inv_freq = (1.0 / (ROPE_BASE ** (np.arange(0, D, 2, dtype=np.float32) / D))).astype(np.float32)
    f2 = np.concatenate([inv_freq, inv_freq]).astype(np.float32)
    ang = pos.astype(np.float32)[None, :] * f2[:, None]          # [128, T]
    cosT = np.cos(ang).astype(np.float32)
    sinT = np.sin(ang).astype(np.float32)

    i = np.arange(128)[:, None]
    j = np.arange(512)[None, :]
    masks = np.stack(
        [np.where(i + 128 * d <= j, np.float32(0.0), np.float32(NEG)) for d in range(4)]
    ).astype(np.float32)                                         # [4, 128, 512]

    nhb = HIST // 64                                             # blocks per seq history
    hist_k = np.asarray(past_key)[bo[:, :nhb]].reshape(B, HIST, NH, D)
    hist_v = np.asarray(past_value)[bo[:, :nhb]].reshape(B, HIST, NH, D)

    wq = w_qkv.reshape(H, NH, 3, D)
    in_maps = []
    for c in range(NCORES):
        hs = slice(c * HPC, (c + 1) * HPC)
        w_qk_c = np.concatenate(
            [wq[:, hs, 0, :].reshape(H, HPC * D), wq[:, hs, 1, :].reshape(H, HPC * D)],
            axis=1)
        w_v_c = wq[:, hs, 2, :].reshape(H, HPC * D)
        wd_c = w_dense[c * HPC * D:(c + 1) * HPC * D, :]
        kTh_c = np.ascontiguousarray(hist_k[:, :, hs, :].transpose(0, 2, 3, 1))
        vh_c = np.ascontiguousarray(hist_v[:, :, hs, :].transpose(0, 2, 1, 3))
        in_maps.append({
            "xT": xT,
            "w_qk": np.ascontiguousarray(w_qk_c),
            "w_v": np.ascontiguousarray(w_v_c),
            "wd": np.ascontiguousarray(wd_c),
            "kTh": kTh_c,
            "vh": vh_c,
            "cosT": cosT,
            "sinT": sinT,
            "masks": masks,
        })
    return in_maps


def kernel(hidden_states, w_qkv, w_dense, past_key, past_value,
           block_offsets, position_ids_1d):
    global _NC
    if _NC is None:
        _NC = _build()
    in_maps = _host_prep(hidden_states, w_qkv, w_dense, past_key, past_value,
                         block_offsets, position_ids_1d)
    res = bass_utils.run_bass_kernel_spmd(_NC, in_maps, core_ids=list(range(NCORES)))
    global _LAST_EXEC_NS
    _LAST_EXEC_NS = getattr(res, "exec_time_ns", None)
    acc = np.zeros((H, T), np.float32)
    for c in range(NCORES):
        acc += res.results[c]["outT"]
    return np.ascontiguousarray(acc.T).reshape(1, T, H).astype(np.float32)
